# revision 1
# baseline (speedup 1.0000x reference)
"""Trainium2 Bass kernel for nn_GATNet (GraphDTA-style GAT network).

Self-contained: host-side sharding/prep + Bass/Tile program + SPMD runner.
Sharding: 8 cores, core c owns nodes [750c, 750c+750) = graphs [16c, 16c+16);
edges partitioned by dst core and sorted by dst; GAT weights replicated;
layer-2 features exchanged via on-device AllGather (fp16 table, f32 logits
bit-packed); MLP head data-parallel.
"""
"""GATNet Trainium kernel: host prep + bass program builder."""
import numpy as np

import concourse.bacc as bacc
import concourse.tile as tile
import concourse.mybir as mybir
from concourse.bass import IndirectOffsetOnAxis
from concourse.masks import make_identity

F32 = mybir.dt.float32
F16 = mybir.dt.float16
I32 = mybir.dt.int32
Alu = mybir.AluOpType
Act = mybir.ActivationFunctionType

N_CORES = 8
N_NODES = 6000
N_GRAPHS = 128
NV = 750
NBLK = 6
GPC = 16
H1, C1 = 10, 1024
D0 = 78
XTW = 96          # xtab row: 0:78 x | 78 one | 79:89 asrc | pad
T2W = 520         # table2 row: 0:512 h2p | 512 one | 513 asrc2 | pad
NEG_SLOPE = 0.2


# ---------------------------------------------------------------- host prep
def prep_edges(ei):
    src = np.concatenate([ei[0], np.arange(N_NODES, dtype=ei.dtype)])
    dst = np.concatenate([ei[1], np.arange(N_NODES, dtype=ei.dtype)])
    order = np.argsort(dst, kind="stable")
    src, dst = src[order], dst[order]
    cores = []
    for c in range(N_CORES):
        m = (dst >= NV * c) & (dst < NV * (c + 1))
        s, d = src[m], dst[m] - NV * c
        blocks = []
        for b in range(NBLK):
            mb = (d >= 128 * b) & (d < min(128 * (b + 1), NV))
            blocks.append((s[mb], d[mb]))
        cores.append(blocks)
    n_ch = [max(1, int(np.ceil(max(len(cores[c][b][0]) for c in range(N_CORES))
                               / 128))) for b in range(NBLK)]
    nch_tot = sum(n_ch)
    src_idx = np.full((N_CORES, 128, nch_tot), 0, np.int32)
    dst_loc = np.full((N_CORES, 128, nch_tot), -1.0, np.float32)
    dst_glo = np.full((N_CORES, 128, nch_tot), NV, np.int32)
    for c in range(N_CORES):
        off = 0
        for b in range(NBLK):
            s, d = cores[c][b]
            npad = n_ch[b] * 128
            sp = np.full(npad, 0, np.int64)
            dp = np.full(npad, -1.0, np.float64)
            gp = np.full(npad, NV, np.int64)
            sp[:len(s)] = s
            dp[:len(d)] = d - 128 * b
            gp[:len(d)] = d
            src_idx[c, :, off:off + n_ch[b]] = sp.reshape(n_ch[b], 128).T
            dst_loc[c, :, off:off + n_ch[b]] = dp.reshape(n_ch[b], 128).T
            dst_glo[c, :, off:off + n_ch[b]] = gp.reshape(n_ch[b], 128).T
            off += n_ch[b]
    return n_ch, src_idx, dst_loc, dst_glo


def host_prep(inputs):
    inp = {k: np.asarray(v) for k, v in inputs.items()}
    Hh = {}
    meta = {}
    W1 = inp["W1"].astype(np.float32)
    W13 = W1.reshape(D0, H1, C1)
    A_s = np.einsum("dhc,hc->dh", W13, inp["a_src1"].astype(np.float32))
    A_d = np.einsum("dhc,hc->dh", W13, inp["a_dst1"].astype(np.float32))
    for k in (1, 2):
        x = inp[f"x{k}"].astype(np.float32)
        asrc = x @ A_s
        adst = x @ A_d
        xtab = np.zeros((N_NODES + 144, XTW), np.float32)
        xtab[:N_NODES, 0:D0] = x
        xtab[:N_NODES, D0] = 1.0
        xtab[:N_NODES, 79:89] = asrc
        Hh[f"xtab{k}"] = xtab
        n_ch, src_idx, dst_loc, dst_glo = prep_edges(inp[f"edge_index{k}"])
        meta[f"n_ch{k}"] = n_ch
        Hh[f"srcidx{k}"] = src_idx
        Hh[f"dstloc{k}"] = dst_loc
        Hh[f"dstglo{k}"] = dst_glo
        nch_tot = sum(n_ch)
        ad = np.zeros((N_CORES, 128, nch_tot * H1), np.float32)
        for c in range(N_CORES):
            dg = np.clip(dst_glo[c].astype(np.int64) + NV * c, 0, N_NODES - 1)
            ad[c] = adst[dg].reshape(128, nch_tot * H1)
        Hh[f"adst1pe{k}"] = ad
        batch = inp[f"batch{k}"]
        bounds = np.searchsorted(batch, np.arange(N_GRAPHS + 1))
        for c in range(N_CORES):
            lb = bounds[GPC * c:GPC * (c + 1) + 1] - NV * c
            assert lb[0] == 0 and lb[-1] == NV, f"graphs not aligned: {lb}"
        meta[f"bounds{k}"] = (bounds[:GPC + 1]).tolist()
    f16 = np.float16
    Hh["W1f"] = W1.astype(f16)
    wvs = (inp["W2"].astype(np.float32) @ inp["a_src2"][0].astype(np.float32))
    wvd = (inp["W2"].astype(np.float32) @ inp["a_dst2"][0].astype(np.float32))
    W2aug = np.zeros((H1 * C1, 516), np.float32)
    W2aug[:, 0:512] = inp["W2"]
    W2aug[:, 512] = wvs
    W2aug[:, 513] = wvd
    Hh["W2aug"] = W2aug.astype(f16)
    Hh["b1"] = inp["b1"].astype(np.float32).reshape(H1 * C1, 1)
    Hh["b2"] = inp["b2"].astype(np.float32).reshape(512, 1)
    for nm, wn, bn, KD, MD in (("g", "Wg", "bg", 512, 128),
                               ("r1", "Wr1", "br1", 954, 2048),
                               ("r2", "Wr2", "br2", 2048, 512),
                               ("r3", "Wr3", "br3", 512, 256),
                               ("f1", "Wf1", "bf1", 512, 1024),
                               ("f2", "Wf2", "bf2", 1024, 512),
                               ("f3", "Wf3", "bf3", 512, 128),
                               ("o", "Wo", "bo", 128, 2)):
        KD0 = inp[wn].shape[0]
        Kp = int(np.ceil(KD0 / 128)) * 128
        W = np.zeros((Kp, MD), f16)
        W[:KD0] = inp[wn].astype(f16)
        Hh[f"W{nm}"] = W
        Hh[f"b{nm}"] = inp[bn].astype(np.float32).reshape(MD, 1)
    cell = inp["cell"].astype(np.float32)
    cT = np.zeros((N_CORES, 1024, GPC), np.float32)
    for c in range(N_CORES):
        cT[c, :954] = cell[GPC * c:GPC * (c + 1)].T
    Hh["cellT"] = cT
    iota = np.broadcast_to(np.arange(128, dtype=np.float32)[None, :],
                           (128, 128)).copy()
    Hh["iotain"] = iota
    return Hh, meta


# ---------------------------------------------------------------- program
def build(Hh, meta, debug_outputs=False, repeat=1, skip_gather=False, skip_cc=False, skip_adg=False):
    nc = bacc.Bacc("TRN2", target_bir_lowering=False, debug=False,
                   num_devices=N_CORES)

    def din(name, arr, dtype):
        return nc.dram_tensor(name, list(arr.shape), dtype,
                              kind="ExternalInput").ap()

    xtab = {k: din(f"xtab{k}", Hh[f"xtab{k}"], F32) for k in (1, 2)}
    W1f = din("W1f", Hh["W1f"], F16)
    W2aug = din("W2aug", Hh["W2aug"], F16)
    b1 = din("b1", Hh["b1"], F32)
    b2 = din("b2", Hh["b2"], F32)
    iotain = din("iotain", Hh["iotain"], F32)
    Wt = {nm: din(f"W{nm}", Hh[f"W{nm}"], F16)
          for nm in ("g", "r1", "r2", "r3", "f1", "f2", "f3", "o")}
    Bt = {nm: din(f"b{nm}", Hh[f"b{nm}"], F32)
          for nm in ("g", "r1", "r2", "r3", "f1", "f2", "f3", "o")}
    percore = {}
    for k in (1, 2):
        for nm, dtp in (("srcidx", I32), ("dstloc", F32), ("dstglo", I32),
                        ("adst1pe", F32)):
            arr = Hh[f"{nm}{k}"]
            percore[f"{nm}{k}"] = nc.dram_tensor(
                f"{nm}{k}", list(arr.shape[1:]), dtp, kind="ExternalInput").ap()
    cellT = nc.dram_tensor("cellT", list(Hh["cellT"].shape[1:]), F32,
                           kind="ExternalInput").ap()

    outT = nc.dram_tensor("outT", [2, GPC], F32, kind="ExternalOutput").ap()
    if debug_outputs:
        dbg_h1T = nc.dram_tensor("dbg_h1T", [128, 768], F32,
                                 kind="ExternalOutput").ap()
        dbg_h2p = nc.dram_tensor("dbg_h2p", [128, T2W], F32,
                                 kind="ExternalOutput").ap()
        dbg_o2T = nc.dram_tensor("dbg_o2T", [128, 768], F32,
                                 kind="ExternalOutput").ap()
        dbg_vT = nc.dram_tensor("dbg_vT", [128, GPC * 2], F32,
                                kind="ExternalOutput").ap()
        dbg_pa = nc.dram_tensor("dbg_pa", [128, 24], F32,
                                kind="ExternalOutput").ap()
        dbg_adtab = nc.dram_tensor("dbg_adtab", [NV + 18, 1], F32,
                                   kind="ExternalOutput").ap()
        dbg_ch0 = nc.dram_tensor("dbg_ch0", [128, T2W + 8], F32,
                                 kind="ExternalOutput").ap()
        dbg_agg2 = nc.dram_tensor("dbg_agg2", [128, 513], F32,
                                  kind="ExternalOutput").ap()
        dbg_sh = nc.dram_tensor("dbg_sh", [128, T2W], F32,
                                kind="ExternalOutput").ap()

    loc2 = {k: nc.dram_tensor(f"loc2_{k}", [NV, T2W], F16).ap() for k in (1, 2)}
    shared2 = {k: nc.dram_tensor(f"shared2_{k}", [N_NODES, T2W], F16,
                                 addr_space="Shared").ap() for k in (1, 2)}
    adtab = {k: nc.dram_tensor(f"adtab_{k}", [NV + 18, 1], F32).ap()
             for k in (1, 2)}

    n_ch = {k: meta[f"n_ch{k}"] for k in (1, 2)}
    nch_tot = {k: sum(n_ch[k]) for k in (1, 2)}
    NCHMAX = max(nch_tot.values())
    bounds = meta["bounds1"]
    assert meta["bounds2"] == bounds

    with tile.TileContext(nc) as tc:
        with (
            tc.tile_pool(name="const", bufs=1) as const,
            tc.tile_pool(name="w1pool", bufs=1) as w1pool,
            tc.tile_pool(name="h1pool", bufs=1) as h1pool,
            tc.tile_pool(name="aggtp", bufs=1) as aggtp,
            tc.tile_pool(name="o2pool", bufs=1) as o2pool,
            tc.tile_pool(name="vpool", bufs=1) as vpool,
            tc.tile_pool(name="sb", bufs=2) as sb,
            tc.tile_pool(name="gp", bufs=3) as gp,
            tc.tile_pool(name="hw", bufs=1) as hw,
        ):
            iota = const.tile([128, 128], F32)
            nc.sync.dma_start(out=iota[:], in_=iotain[:])
            ident = const.tile([128, 128], F32)
            make_identity(nc, ident)
            zer = const.tile([128, 1], F32)
            nc.vector.memset(zer[:], 0.0)

            w1t = w1pool.tile([D0, H1 * C1], F16)
            nc.gpsimd.dma_start(out=w1t[:], in_=W1f[:])
            b1t = const.tile([128, H1 * C1 // 128], F32)
            nc.sync.dma_start(out=b1t[:],
                              in_=b1.rearrange("(a p) o -> p (a o)", p=128))
            b2t = const.tile([128, 4], F32)
            nc.sync.dma_start(out=b2t[:],
                              in_=b2.rearrange("(a p) o -> p (a o)", p=128))

            h1T = [h1pool.tile([128, 768], F16, tag=f"h1T{i}", name=f"h1T{i}") for i in range(80)]
            o2T = [o2pool.tile([128, 768], F16, tag=f"o2T{i}", name=f"o2T{i}") for i in range(4)]
            vT = {k: vpool.tile([128, GPC], F16, tag=f"vT{k}", name=f"vT{k}") for k in (1, 2)}
            aggT = [aggtp.tile([D0, 768], F16, tag=f"aggT{h}", name=f"aggT{h}")
                    for h in range(H1)]

            def lrelu_exp(src_ap, n, tag):
                t1 = sb.tile([128, n], F32, tag=f"{tag}_t1")
                nc.vector.tensor_scalar_mul(t1[:], src_ap, NEG_SLOPE)
                t2 = sb.tile([128, n], F32, tag=f"{tag}_t2")
                nc.vector.tensor_tensor(out=t2[:], in0=t1[:], in1=src_ap,
                                        op=Alu.max)
                w = sb.tile([128, n], F32, tag=f"{tag}_w")
                nc.scalar.activation(w[:], t2[:], Act.Exp)
                return w

            def elu_psum(ps_ap, bias_col, n, out_ap, alt=0):
                x = sb.tile([128, n], F32, tag="elu_x")
                nc.vector.tensor_scalar(out=x[:], in0=ps_ap, scalar1=bias_col,
                                        scalar2=None, op0=Alu.add)
                ex = sb.tile([128, n], F32, tag="elu_e")
                nc.scalar.activation(ex[:], x[:], Act.Exp)
                # ex <- min(ex - 1, 0)  (in place)
                nc.vector.tensor_scalar(out=ex[:], in0=ex[:], scalar1=-1.0,
                                        scalar2=0.0, op0=Alu.add, op1=Alu.min)
                # x <- max(x, 0)  (in place)
                if alt:
                    nc.scalar.activation(x[:], x[:], Act.Relu)
                else:
                    nc.vector.tensor_scalar_max(x[:], x[:], 0.0)
                nc.vector.tensor_tensor(out=out_ap, in0=x[:], in1=ex[:],
                                        op=Alu.add)

            def _body():
                for k in (1, 2):
                    NCH = nch_tot[k]
                    sit = const.tile([128, NCH], I32, tag=f"sit{k}")
                    nc.sync.dma_start(out=sit[:], in_=percore[f"srcidx{k}"][:])
                    dlt = const.tile([128, NCH], F32, tag=f"dlt{k}")
                    nc.sync.dma_start(out=dlt[:], in_=percore[f"dstloc{k}"][:])
                    dgt = const.tile([128, NCH], I32, tag=f"dgt{k}")
                    nc.sync.dma_start(out=dgt[:], in_=percore[f"dstglo{k}"][:])
                    adpe = const.tile([128, NCH * H1], F32, tag=f"adpe{k}")
                    nc.sync.dma_start(out=adpe[:], in_=percore[f"adst1pe{k}"][:])
                    nc.sync.dma_start(out=adtab[k][NV:NV + 18, :],
                                      in_=zer[0:18, :].broadcast_to([18, 1]))

                    # ---------- L1: attention + aggregation ----------
                    with (
                        tc.tile_pool(name=f"psagg{k}", bufs=2, space="PSUM") as psagg,
                        tc.tile_pool(name=f"pstr{k}", bufs=2, space="PSUM") as pstr,
                    ):
                        ch0 = 0
                        for b in range(NBLK):
                            nchb = n_ch[k][b]
                            ps = psagg.tile([128, H1 * 79], F32, tag="agg")
                            for ci in range(nchb):
                                ch = ch0 + ci
                                xg = gp.tile([128, XTW], F32, tag="xg")
                                if skip_gather:
                                    nc.sync.dma_start(out=xg[:], in_=xtab[k][(ch % 40) * 128:(ch % 40) * 128 + 128, :])
                                else:
                                    nc.gpsimd.indirect_dma_start(
                                        out=xg[:], out_offset=None, in_=xtab[k],
                                        in_offset=IndirectOffsetOnAxis(
                                            ap=sit[:, ch:ch + 1], axis=0))
                                e0 = gp.tile([128, H1], F32, tag="e0")
                                nc.vector.tensor_tensor(
                                    out=e0[:], in0=xg[:, 79:89],
                                    in1=adpe[:, ch * H1:(ch + 1) * H1], op=Alu.add)
                                w = lrelu_exp(e0[:], H1, "l1")
                                oh = sb.tile([128, 128], F16, tag="oh")
                                nc.vector.tensor_scalar(
                                    out=oh[:], in0=iota[:],
                                    scalar1=dlt[:, ch:ch + 1], scalar2=None,
                                    op0=Alu.is_equal)
                                wxg = sb.tile([128, H1, 79], F16, tag="wxg")
                                nc.vector.tensor_tensor(
                                    out=wxg[:],
                                    in0=xg[:, 0:79].rearrange(
                                        "p (o f) -> p o f", o=1).broadcast_to(
                                            [128, H1, 79]),
                                    in1=w[:].rearrange(
                                        "p (h o) -> p h o", o=1).broadcast_to(
                                            [128, H1, 79]),
                                    op=Alu.mult)
                                wxg2 = wxg[:].rearrange("p h f -> p (h f)")
                                nc.tensor.matmul(ps[:, 0:512], oh[:],
                                                 wxg2[:, 0:512],
                                                 start=(ci == 0), stop=False)
                                nc.tensor.matmul(ps[:, 512:790], oh[:],
                                                 wxg2[:, 512:790],
                                                 start=(ci == 0),
                                                 stop=(ci == nchb - 1))
                            den = sb.tile([128, H1], F32, tag="den")
                            nc.vector.tensor_scalar_max(
                                den[:],
                                ps[:].rearrange("p (h f) -> p h f", f=79)[:, :, 78],
                                1e-30)
                            rec = sb.tile([128, H1], F32, tag="rec")
                            nc.vector.reciprocal(out=rec[:], in_=den[:])
                            for h in range(H1):
                                sc = sb.tile([128, D0], F32, tag="sc")
                                nc.vector.tensor_scalar(
                                    out=sc[:], in0=ps[:, h * 79:h * 79 + D0],
                                    scalar1=rec[:, h:h + 1], scalar2=None,
                                    op0=Alu.mult)
                                tp = pstr.tile([D0, 128], F32, tag="tp")
                                nc.tensor.transpose(out=tp[:], in_=sc[:],
                                                    identity=ident[:])
                                nc.scalar.copy(
                                    out=aggT[h][:, b * 128:(b + 1) * 128],
                                    in_=tp[:])
                            ch0 += nchb

                    # ---------- L1 finalize ----------
                    with tc.tile_pool(name=f"psfin{k}", bufs=2,
                                      space="PSUM") as psfin:
                        for h in range(H1):
                            for cc in range(8):
                                i = h * 8 + cc
                                pf = psfin.tile([128, 768], F32, tag="fin")
                                lhs = w1t[:, i * 128:(i + 1) * 128]
                                nc.tensor.matmul(pf[:, 0:512], lhs,
                                                 aggT[h][:, 0:512],
                                                 start=True, stop=False)
                                nc.tensor.matmul(pf[:, 512:768], lhs,
                                                 aggT[h][:, 512:768],
                                                 start=True, stop=True)
                                elu_psum(pf[:], b1t[:, i:i + 1], 768,
                                         h1T[i][:], alt=cc % 2)
                        if debug_outputs and k == 1:
                            nc.gpsimd.dma_start(out=dbg_h1T[:], in_=h1T[0][:])

                    # ---------- L2 big matmul ----------
                    with (
                        tc.tile_pool(name=f"psmm{k}", bufs=1, space="PSUM") as psmm,
                        tc.tile_pool(name=f"psaux{k}", bufs=1, space="PSUM") as psaux,
                    ):
                        pm = [psmm.tile([128, 512], F32, tag=f"pm{m}", name=f"pm{m}")
                              for m in range(6)]
                        pa = psaux.tile([128, 24], F32, tag="pa")
                        for kc in range(80):
                            wchunk = sb.tile([128, 516], F16, tag="w2c")
                            nc.sync.dma_start(
                                out=wchunk[:],
                                in_=W2aug[kc * 128:(kc + 1) * 128, :])
                            for m in range(6):
                                lhs = h1T[kc][:, m * 128:(m + 1) * 128]
                                nc.tensor.matmul(pm[m][:], lhs, wchunk[:, 0:512],
                                                 start=(kc == 0), stop=(kc == 79))
                                nc.tensor.matmul(pa[:, m * 4:m * 4 + 4], lhs,
                                                 wchunk[:, 512:516],
                                                 start=(kc == 0 and m == 0),
                                                 stop=(kc == 79),
                                                 skip_group_check=True)
                        if debug_outputs and k == 1:
                            pa_s = hw.tile([128, 24], F32, tag="pa_s")
                            nc.vector.tensor_copy(out=pa_s[:], in_=pa[:])
                            nc.sync.dma_start(out=dbg_pa[:], in_=pa_s[:])
                        for m in range(6):
                            nrow = 128 if m < 5 else NV - 640
                            loc = sb.tile([128, T2W], F16, tag="loc")
                            nc.scalar.copy(out=loc[:, 0:512], in_=pm[m][:])
                            nc.vector.memset(loc[:, 512:513], 1.0)
                            nc.vector.memset(loc[:, 513:514], 0.0)
                            nc.vector.tensor_copy(
                                out=loc[:, 514:516].bitcast(F32),
                                in_=pa[:, m * 4:m * 4 + 1])
                            nc.vector.memset(loc[:, 516:T2W], 0.0)
                            nc.sync.dma_start(
                                out=loc2[k][m * 128:m * 128 + nrow, :],
                                in_=loc[0:nrow, :])
                            ad = sb.tile([128, 1], F32, tag="adcol")
                            nc.vector.tensor_copy(out=ad[:],
                                                  in_=pa[:, m * 4 + 1:m * 4 + 2])
                            nc.sync.dma_start(
                                out=adtab[k][m * 128:m * 128 + nrow, :],
                                in_=ad[0:nrow, :])
                            if debug_outputs and k == 1 and m == 0:
                                nc.sync.dma_start(out=dbg_h2p[:], in_=loc[:])

                    if skip_cc:
                        nc.sync.dma_start(out=shared2[k][0:NV, :], in_=loc2[k][:])
                    else:
                        nc.gpsimd.collective_compute(
                            "AllGather", Alu.bypass,
                            replica_groups=[list(range(N_CORES))],
                            ins=[loc2[k][:].opt()], outs=[shared2[k][:].opt()])

                    if debug_outputs and k == 1:
                        nc.sync.dma_start(out=dbg_adtab[:], in_=adtab[k][:])
                        shs = hw.tile([128, T2W], F32, tag="shs")
                        nc.sync.dma_start(out=shs[:], in_=shared2[k][3000:3128, :])
                        nc.sync.dma_start(out=dbg_sh[:], in_=shs[:])
                    # ---------- L2 attention + aggregation ----------
                    with (
                        tc.tile_pool(name=f"psag2{k}", bufs=2, space="PSUM") as psag2,
                        tc.tile_pool(name=f"pstr2{k}", bufs=2, space="PSUM") as pstr2,
                    ):
                        ch0 = 0
                        for b in range(NBLK):
                            nchb = n_ch[k][b]
                            ps = psag2.tile([128, 513], F32, tag="agg2")
                            for ci in range(nchb):
                                ch = ch0 + ci
                                hg = gp.tile([128, T2W], F16, tag="hg")
                                if skip_gather:
                                    nc.sync.dma_start(out=hg[:], in_=shared2[k][(ch % 40) * 128:(ch % 40) * 128 + 128, :])
                                else:
                                    nc.gpsimd.indirect_dma_start(
                                        out=hg[:], out_offset=None, in_=shared2[k],
                                        in_offset=IndirectOffsetOnAxis(
                                            ap=sit[:, ch:ch + 1], axis=0))
                                adg = gp.tile([128, 1], F32, tag="adg")
                                if skip_gather or skip_adg:
                                    nc.vector.memset(adg[:], 0.0)
                                else:
                                    nc.gpsimd.indirect_dma_start(
                                        out=adg[:], out_offset=None, in_=adtab[k],
                                        in_offset=IndirectOffsetOnAxis(
                                            ap=dgt[:, ch:ch + 1], axis=0))
                                e0 = gp.tile([128, 1], F32, tag="e2")
                                nc.vector.tensor_tensor(
                                    out=e0[:],
                                    in0=hg[:, 514:516].bitcast(F32),
                                    in1=adg[:], op=Alu.add)
                                w2 = lrelu_exp(e0[:], 1, "l2")
                                if debug_outputs and k == 1 and ch == 0:
                                    d0 = hw.tile([128, T2W + 8], F32, tag="dch0")
                                    nc.vector.tensor_copy(out=d0[:, 0:T2W], in_=hg[:])
                                    nc.vector.tensor_copy(out=d0[:, T2W:T2W + 1], in_=adg[:])
                                    nc.vector.tensor_copy(out=d0[:, T2W + 1:T2W + 2], in_=e0[:])
                                    nc.vector.tensor_copy(out=d0[:, T2W + 2:T2W + 3], in_=w2[:])
                                    nc.vector.memset(d0[:, T2W + 3:T2W + 8], 0.0)
                                    nc.sync.dma_start(out=dbg_ch0[:], in_=d0[:])
                                wh = gp.tile([128, 513], F16, tag="wh")
                                nc.vector.tensor_scalar(
                                    out=wh[:], in0=hg[:, 0:513],
                                    scalar1=w2[:, 0:1], scalar2=None, op0=Alu.mult)
                                oh = sb.tile([128, 128], F16, tag="oh")
                                nc.vector.tensor_scalar(
                                    out=oh[:], in0=iota[:],
                                    scalar1=dlt[:, ch:ch + 1], scalar2=None,
                                    op0=Alu.is_equal)
                                nc.tensor.matmul(ps[:, 0:512], oh[:],
                                                 wh[:, 0:512],
                                                 start=(ci == 0), stop=False)
                                nc.tensor.matmul(ps[:, 512:513], oh[:],
                                                 wh[:, 512:513],
                                                 start=(ci == 0),
                                                 stop=(ci == nchb - 1))
                            if debug_outputs and k == 1 and b == 0:
                                ag_s = hw.tile([128, 513], F32, tag="ag_s")
                                nc.vector.tensor_copy(out=ag_s[:], in_=ps[:])
                                nc.sync.dma_start(out=dbg_agg2[:], in_=ag_s[:])
                            den = sb.tile([128, 1], F32, tag="dn2")
                            nc.vector.tensor_scalar_max(den[:], ps[:, 512:513],
                                                        1e-30)
                            rec = sb.tile([128, 1], F32, tag="rc2")
                            nc.vector.reciprocal(out=rec[:], in_=den[:])
                            for cc in range(4):
                                sc = sb.tile([128, 128], F32, tag="sc2")
                                nc.vector.tensor_scalar(
                                    out=sc[:], in0=ps[:, cc * 128:(cc + 1) * 128],
                                    scalar1=rec[:, 0:1], scalar2=None, op0=Alu.mult)
                                tp = pstr2.tile([128, 128], F32, tag="tp2")
                                nc.tensor.transpose(out=tp[:], in_=sc[:],
                                                    identity=ident[:])
                                elu_psum(tp[:], b2t[:, cc:cc + 1], 128,
                                         o2T[cc][:, b * 128:(b + 1) * 128],
                                         alt=cc % 2)
                            ch0 += nchb
                        if debug_outputs and k == 1:
                            nc.gpsimd.dma_start(out=dbg_o2T[:], in_=o2T[0][:])

                    # ---------- maxpool + Wg ----------
                    with tc.tile_pool(name=f"psg{k}", bufs=2, space="PSUM") as psg:
                        gT = hw.tile([128, 4, GPC], F16, tag="gT")
                        for cc in range(4):
                            for g in range(GPC):
                                nc.vector.tensor_reduce(
                                    out=gT[:, cc, g:g + 1],
                                    in_=o2T[cc][:, bounds[g]:bounds[g + 1]],
                                    axis=mybir.AxisListType.X, op=Alu.max)
                        pg = psg.tile([128, GPC], F32, tag="pg")
                        for kc in range(4):
                            wgt = sb.tile([128, 128], F16, tag="dw")
                            nc.sync.dma_start(
                                out=wgt[:], in_=Wt["g"][kc * 128:(kc + 1) * 128, :])
                            nc.tensor.matmul(pg[:], wgt[:], gT[:, kc, :],
                                             start=(kc == 0), stop=(kc == 3))
                        bgt = sb.tile([128, 1], F32, tag="bcol")
                        nc.sync.dma_start(out=bgt[:], in_=Bt["g"][:])
                        nc.scalar.activation(vT[k][:], pg[:], Act.Relu,
                                             bias=bgt[:, 0:1])

                if debug_outputs:
                    vt1 = hw.tile([128, GPC], F32, tag="vdbg1")
                    nc.vector.tensor_copy(out=vt1[:], in_=vT[1][:])
                    nc.sync.dma_start(out=dbg_vT[:, 0:GPC], in_=vt1[:])
                    vt2 = hw.tile([128, GPC], F32, tag="vdbg2")
                    nc.vector.tensor_copy(out=vt2[:], in_=vT[2][:])
                    nc.sync.dma_start(out=dbg_vT[:, GPC:2 * GPC], in_=vt2[:])

                # ---------- head ----------
                def l2norm_scale(xtiles, tag):
                    n = len(xtiles)
                    with tc.tile_pool(name=f"psn{tag}", bufs=1, space="PSUM") as psn:
                        pn = psn.tile([1, GPC], F32, tag=f"pn{tag}")
                        ones = const.tile([128, 1], F16, tag=f"one{tag}")
                        nc.vector.memset(ones[:], 1.0)
                        for i in range(n):
                            sq = sb.tile([128, GPC], F16, tag=f"sq{tag}")
                            nc.scalar.activation(sq[:], xtiles[i][:], Act.Square)
                            nc.tensor.matmul(pn[:], ones[:], sq[:],
                                             start=(i == 0), stop=(i == n - 1))
                        nrm = sb.tile([1, GPC], F32, tag=f"nr{tag}")
                        nc.scalar.activation(nrm[:], pn[:], Act.Sqrt)
                        nc.vector.tensor_scalar_max(nrm[:], nrm[:], 1e-12)
                        rcp = sb.tile([1, GPC], F32, tag=f"rcn{tag}")
                        nc.vector.reciprocal(out=rcp[:], in_=nrm[:])
                        rb = hw.tile([128, GPC], F32, tag=f"rb{tag}")
                        nc.gpsimd.partition_broadcast(rb[:], rcp[:])
                        outs = []
                        for i in range(n):
                            o = hw.tile([128, GPC], F16, tag=f"no{tag}{i}")
                            nc.vector.tensor_tensor(out=o[:], in0=xtiles[i][:],
                                                    in1=rb[:], op=Alu.mult)
                            outs.append(o)
                        return outs

                def dense(xtiles, nm, md, act=True, out_f32=False):
                    kc = len(xtiles)
                    mc = (md + 127) // 128
                    outs = []
                    bt = sb.tile([min(128, md), (md + 127) // 128], F32,
                                 tag=f"bt{nm}")
                    nc.sync.dma_start(
                        out=bt[:],
                        in_=Bt[nm].rearrange("(a p) o -> p (a o)",
                                             p=min(128, md)))
                    with tc.tile_pool(name=f"psd{nm}", bufs=2, space="PSUM") as psd:
                        for m in range(mc):
                            mw = min(128, md - m * 128)
                            pd = psd.tile([mw, GPC], F32, tag=f"pd{nm}")
                            for i in range(kc):
                                dw = sb.tile([128, mw], F16, tag="dw")
                                nc.sync.dma_start(
                                    out=dw[:],
                                    in_=Wt[nm][i * 128:(i + 1) * 128,
                                               m * 128:m * 128 + mw])
                                nc.tensor.matmul(pd[:], dw[:], xtiles[i][:],
                                                 start=(i == 0), stop=(i == kc - 1))
                            o = hw.tile([mw, GPC], F32 if out_f32 else F16,
                                        tag=f"do{nm}{m}")
                            if act:
                                nc.scalar.activation(o[:], pd[:], Act.Relu,
                                                     bias=bt[0:mw, m:m + 1])
                            else:
                                nc.vector.tensor_scalar(out=o[:], in0=pd[:],
                                                        scalar1=bt[0:mw, m:m + 1],
                                                        scalar2=None, op0=Alu.add)
                            outs.append(o)
                    return outs

                cT_t = []
                for i in range(8):
                    t = hw.tile([128, GPC], F32, tag=f"cT{i}")
                    nc.sync.dma_start(out=t[:], in_=cellT[i * 128:(i + 1) * 128, :])
                    cT_t.append(t)
                cn = l2norm_scale(cT_t, "c")
                r1 = dense(cn, "r1", 2048)
                r2 = dense(r1, "r2", 512)
                r3 = dense(r2, "r3", 256)
                xc_t = []
                for j, src_t in enumerate((vT[1], vT[2], r3[0], r3[1])):
                    t = hw.tile([128, GPC], F32, tag=f"xc{j}")
                    nc.vector.tensor_copy(out=t[:], in_=src_t[:])
                    xc_t.append(t)
                xn = l2norm_scale(xc_t, "x")
                f1 = dense(xn, "f1", 1024)
                f2 = dense(f1, "f2", 512)
                f3 = dense(f2, "f3", 128)
                fo = dense(f3, "o", 2, act=False, out_f32=True)
                nc.sync.dma_start(out=outT[:], in_=fo[0][:])
            for _rep in range(repeat):
                _body()

    nc.compile()
    return nc


def make_in_maps(Hh):
    ins = []
    for c in range(N_CORES):
        m = {}
        for k, v in Hh.items():
            if k in ("srcidx1", "dstloc1", "dstglo1", "adst1pe1",
                     "srcidx2", "dstloc2", "dstglo2", "adst1pe2", "cellT"):
                m[k] = np.ascontiguousarray(v[c])
            else:
                m[k] = v
        ins.append(m)
    return ins


# ---------------------------------------------------------------------- runner
import time
import jax
from jax.sharding import Mesh, PartitionSpec
from jax.experimental.shard_map import shard_map
from concourse import bass2jax
from concourse.bass2jax import _bass_exec_p, install_neuronx_cc_hook

import time
import numpy as np
import jax
from jax.sharding import Mesh, PartitionSpec
from jax.experimental.shard_map import shard_map

import concourse.mybir as mybir
from concourse import bass2jax
from concourse.bass2jax import _bass_exec_p, install_neuronx_cc_hook


class SpmdRunner:
    def __init__(self, nc, n_cores: int):
        install_neuronx_cc_hook()
        self.nc = nc
        self.n_cores = n_cores
        partition_name = nc.partition_id_tensor.name if nc.partition_id_tensor else None
        in_names, out_names, out_avals, zero_outs = [], [], [], []
        for alloc in nc.m.functions[0].allocations:
            if not isinstance(alloc, mybir.MemoryLocationSet):
                continue
            name = alloc.memorylocations[0].name
            if alloc.kind == "ExternalInput":
                if name != partition_name:
                    in_names.append(name)
            elif alloc.kind == "ExternalOutput":
                out_names.append(name)
                shape = tuple(alloc.tensor_shape)
                dtype = mybir.dt.np(alloc.dtype)
                out_avals.append(jax.core.ShapedArray(shape, dtype))
                zero_outs.append(np.zeros(shape, dtype))
        self.in_names = list(in_names)
        self.out_names = out_names
        self.out_avals = out_avals
        self.zero_outs = zero_outs
        n_params = len(in_names)
        self.n_params = n_params
        all_in_names = list(in_names) + list(out_names)
        if partition_name is not None:
            all_in_names.append(partition_name)

        def _body(*args):
            operands = list(args)
            if partition_name is not None:
                operands.append(bass2jax.partition_id_tensor())
            outs = _bass_exec_p.bind(
                *operands,
                out_avals=tuple(out_avals),
                in_names=tuple(all_in_names),
                out_names=tuple(out_names),
                lowering_input_output_aliases=(),
                sim_require_finite=True,
                sim_require_nnan=True,
                nc=nc,
            )
            return tuple(outs)

        donate = tuple(range(n_params, n_params + len(out_names)))
        devices = jax.devices()[:n_cores]
        mesh = Mesh(np.asarray(devices), ("core",))
        in_specs = (PartitionSpec("core"),) * (n_params + len(out_names))
        out_specs = (PartitionSpec("core"),) * len(out_names)
        self._fn = jax.jit(
            shard_map(_body, mesh=mesh, in_specs=in_specs, out_specs=out_specs,
                      check_rep=False),
            donate_argnums=donate, keep_unused=True)

    def _concat_inputs(self, in_maps):
        per_core = [[np.asarray(m[n]) for n in self.in_names] for m in in_maps]
        return [np.concatenate([per_core[c][i] for c in range(self.n_cores)], axis=0)
                for i in range(self.n_params)]

    def _zeros(self):
        return [np.zeros((self.n_cores * z.shape[0], *z.shape[1:]), z.dtype)
                for z in self.zero_outs]

    def run(self, in_maps):
        concat_in = self._concat_inputs(in_maps)
        outs = self._fn(*concat_in, *self._zeros())
        res = []
        for c in range(self.n_cores):
            d = {}
            for i, name in enumerate(self.out_names):
                d[name] = np.asarray(outs[i]).reshape(
                    self.n_cores, *self.out_avals[i].shape)[c]
            res.append(d)
        return res

    def time(self, in_maps, iters=20, warmup=3, inner=5):
        """Returns (best_per_call_s, all_times). Dispatches `inner` calls
        back-to-back then blocks, to amortize host->terminal latency."""
        concat_in = [jax.device_put(x) for x in self._concat_inputs(in_maps)]
        times = []
        for it in range(warmup + iters):
            zs = [self._zeros() for _ in range(inner)]
            t0 = time.perf_counter()
            outs = None
            for k in range(inner):
                outs = self._fn(*concat_in, *zs[k])
            jax.block_until_ready(outs)
            dt = (time.perf_counter() - t0) / inner
            if it >= warmup:
                times.append(dt)
        return min(times), times


# ---------------------------------------------------------------- entry point
_CACHE = {}


def _get_runner(Hh, meta):
    key = (tuple(meta["n_ch1"]), tuple(meta["n_ch2"]), tuple(meta["bounds1"]))
    ent = _CACHE.get(key)
    if ent is None:
        nc = build(Hh, meta)
        ent = SpmdRunner(nc, N_CORES)
        _CACHE[key] = ent
    return ent


def kernel(**inputs):
    Hh, meta = host_prep(inputs)
    runner = _get_runner(Hh, meta)
    res = runner.run(make_in_maps(Hh))
    out = np.concatenate([res[c]["outT"].T for c in range(N_CORES)], axis=0)
    return out.astype(np.float32)

